# revision 1
# baseline (speedup 1.0000x reference)
"""Trainium2 Bass kernel for the batched 2D Kalman filter (nn_KalmanFilterWrapper).

Math
----
The reference runs, per trajectory, a Kalman filter over T=4096 steps with a
constant-velocity model.  The gain/covariance recursion (Riccati) is
data-independent, so the scan collapses to a linear time-varying recurrence

    x_t = A_t x_{t-1} + k_t z_t,        y_t = x_t[0]

with coefficients shared across the whole batch.  The 4-state filter decouples
into two identical 2-state (position, velocity) scalar filters — one per
coordinate — giving B*2 = 8192 independent scalar sequences.

Blocking time into chunks of C=126 steps turns the whole filter into one
[128x128] @ [128xN] matmul per block: the contraction covers the block's 126
measurements plus 2 "carry" rows holding the filter state from the previous
block; output rows are the block's 126 positions plus duplicated (p_last,
v_last) rows that become the next block's carry.  All coefficient matrices are
precomputed on the host in float64.

Partition layout (all compute-engine accesses start at partition 0/64):
  contract rows: 0..1 = carry (p_prev, v_prev), 2+j = z_j
  output rows:   0 = p_last (dup), 1 = v_last, 2+j = p_j
The last (short, 64-step) block reads the final 126 input rows with zero
coefficients on the first 62, so no memset/padding is needed.

Sharding: data-parallel across 8 NeuronCores, 512 trajectories (1024 scalar
sequences) per core.  Layout on device is [time, sequence]; the host
transposes in/out of the reference's [batch, time, 2] layout.
"""

import numpy as np

import concourse.bass as bass
import concourse.bacc as bacc
import concourse.mybir as mybir
from concourse.bass_utils import run_bass_kernel_spmd
from concourse.tile import TileContext

# Problem constants (hardcoded per harness contract).
B = 4096
T = 4096
DT = 1.0
PROCESS_VARIANCE = 1e-05
MEASUREMENT_VARIANCE = 0.1
INIT_ERROR = 1.0

N_CORES = 8
NCOLS = (B * 2) // N_CORES  # 1024 scalar sequences per core
MAIN_C = 126                # block size; contract dim = C + 2 = 128
CHUNK = 512                 # matmul moving free-dim (fp32 max, one PSUM bank)

DT_F32 = mybir.dt.float32
USE_F32R = False  # fp32r: full-rate PE matmul (vs 2-pass fp32), ~1e-4 rel err
DT_F32R = mybir.dt.float32r if USE_F32R else mybir.dt.float32


def _blocks():
    """Returns [(t0_dma, n_skip)]; each block reads z[t0_dma : t0_dma+126] and
    filters steps t0_dma+n_skip .. t0_dma+125 (n_skip leading rows get zero
    coefficients)."""
    out = []
    t0 = 0
    while t0 + MAIN_C <= T:
        out.append((t0, 0))
        t0 += MAIN_C
    if t0 < T:
        rem = T - t0
        out.append((T - MAIN_C, MAIN_C - rem))
    return out


def _precompute_lhsT():
    """Host-side Riccati + per-block coefficient matrices, float64 -> f32.

    Returns [128, n_blocks*128] f32; block bi's stationary operand (lhsT) is
    cols [bi*128, (bi+1)*128): lhsT[k_contract, m_out] = U[m, k].
    """
    F = np.array([[1.0, DT], [0.0, 1.0]], dtype=np.float64)
    I2 = np.eye(2, dtype=np.float64)
    P = INIT_ERROR * I2.copy()
    A = np.zeros((T, 2, 2), dtype=np.float64)
    k = np.zeros((T, 2), dtype=np.float64)
    for t in range(T):
        Pp = F @ P @ F.T + PROCESS_VARIANCE * I2
        s = Pp[0, 0] + MEASUREMENT_VARIANCE
        kt = Pp[:, 0] / s
        k[t] = kt
        KH = np.zeros((2, 2), dtype=np.float64)
        KH[:, 0] = kt
        P = (I2 - KH) @ Pp
        A[t] = (I2 - KH) @ F

    blocks = _blocks()
    lhsT_all = np.zeros((128, len(blocks) * 128), dtype=np.float64)
    for bi, (t0, n_skip) in enumerate(blocks):
        # contract col of z_j within this block's tile: block 0 loads z[0:128]
        # into partitions 0..127 (no carry), others load z into 2..127.
        zcol = (lambda j: j) if bi == 0 else (lambda j: 2 + j)
        Rc = np.zeros((2, 128), dtype=np.float64)
        if bi == 0:
            Rc[0, 0] = 1.0  # x_{-1} = [z_0, 0]
        else:
            Rc[0, 0] = 1.0  # carry row 0 = p_prev
            Rc[1, 1] = 1.0  # carry row 1 = v_prev
        U = np.zeros((128, 128), dtype=np.float64)
        for j in range(MAIN_C):
            if j >= n_skip:
                t = t0 + j
                Rc = A[t] @ Rc
                Rc[:, zcol(j)] += k[t]
            U[2 + j, :] = Rc[0, :]
        U[0, :] = Rc[0, :]  # p_last (dup) -> next block carry row 0
        U[1, :] = Rc[1, :]  # v_last      -> next block carry row 1
        lhsT_all[:, bi * 128:(bi + 1) * 128] = U.T
    return np.ascontiguousarray(lhsT_all.astype(np.float32))


def _build_nc():
    blocks = _blocks()
    nblk = len(blocks)
    nchunks = NCOLS // CHUNK

    nc = bacc.Bacc()
    z = nc.dram_tensor("z", [T, NCOLS], DT_F32R, kind="ExternalInput")
    u = nc.dram_tensor("u", [128, nblk * 128], DT_F32R, kind="ExternalInput")
    v = nc.dram_tensor("v", [T, NCOLS], DT_F32, kind="ExternalOutput")

    # Group blocks into DMA units: blocks 0 and 1 are singles (block 0 has the
    # special full-128-row load), then pairs, then the short last block single.
    # Paired units move 1 MB per dma_start instead of 516 KB.
    units = [[0], [1]]
    bi = 2
    while bi + 1 < nblk - 1:
        units.append([bi, bi + 1])
        bi += 2
    while bi < nblk:
        units.append([bi])
        bi += 1

    with TileContext(nc) as tc:
        with (
            tc.tile_pool(name="consts", bufs=1) as cpool,
            tc.tile_pool(name="zpool", bufs=4) as zpool,
            tc.tile_pool(name="vpool", bufs=3) as vpool,
            tc.tile_pool(name="psum", bufs=4, space="PSUM") as ppool,
        ):
            u_tile = cpool.tile([128, nblk * 128], DT_F32R)
            nc.sync.dma_start(u_tile[:, :], u[:, :])

            # z tiles per unit; ztile[bi] = (tile, sub-index)
            ztile = {}
            for unit in units:
                zp = zpool.tile([128, 2, NCOLS], DT_F32R, tag="zp")
                if len(unit) == 2:
                    t0 = blocks[unit[0]][0]
                    src = z[t0:t0 + 2 * MAIN_C, :].rearrange(
                        "(b r) c -> r b c", b=2
                    )
                    nc.sync.dma_start(zp[2:128, :, :], src)
                elif unit[0] == 0:
                    # block 0 has no carry: load z[0:128] into all partitions
                    # (rows 126..127 get zero coefficients) — no memset needed
                    nc.sync.dma_start(zp[:, 0, :], z[0:128, :])
                else:
                    t0 = blocks[unit[0]][0]
                    nc.sync.dma_start(zp[2:128, 0, :], z[t0:t0 + MAIN_C, :])
                for si, b in enumerate(unit):
                    ztile[b] = (zp, si)

            for unit in units:
                vout = vpool.tile([128, 2, NCOLS], DT_F32, tag="vout")
                for si, b in enumerate(unit):
                    zp, zsub = ztile[b]
                    for ci in range(nchunks):
                        cols = bass.ds(ci * CHUNK, CHUNK)
                        ps = ppool.tile([128, CHUNK], DT_F32)
                        nc.tensor.matmul(
                            ps[:, :],
                            u_tile[:, bass.ds(b * 128, 128)],
                            zp[:, zsub, cols],
                            start=True,
                            stop=True,
                        )
                        if b + 1 < nblk:
                            zn, nsub = ztile[b + 1]
                            nc.scalar.copy(zn[0:2, nsub, cols], ps[0:2, :])
                        # split evictions across ACT and DVE
                        if ci % 2 == 0:
                            nc.scalar.copy(vout[:, si, cols], ps[:, :])
                        else:
                            nc.vector.tensor_copy(vout[:, si, cols], ps[:, :])
                if len(unit) == 2:
                    t0 = blocks[unit[0]][0]
                    dst = v[t0:t0 + 2 * MAIN_C, :].rearrange(
                        "(b r) c -> r b c", b=2
                    )
                    nc.sync.dma_start(dst, vout[2:128, :, :])
                else:
                    t0, n_skip = blocks[unit[0]]
                    nc.sync.dma_start(
                        v[t0 + n_skip:t0 + MAIN_C, :],
                        vout[2 + n_skip:128, 0, :],
                    )
    nc.finalize()  # Bacc.compile(): splits multi-waits, allocates registers
    return nc


_CACHE = {}


def _run(x_seq: np.ndarray, trace: bool = False):
    if "nc" not in _CACHE:
        _CACHE["nc"] = _build_nc()
        _CACHE["u"] = _precompute_lhsT()
    nc = _CACHE["nc"]
    u_all = _CACHE["u"]

    x = np.ascontiguousarray(np.asarray(x_seq, dtype=np.float32))
    assert x.shape == (B, T, 2), x.shape

    # [B, T, 2] -> [T, B*2]; column n = 2*b + c
    zt = np.ascontiguousarray(x.transpose(1, 0, 2).reshape(T, B * 2))

    in_maps = [
        {"z": np.ascontiguousarray(zt[:, i * NCOLS:(i + 1) * NCOLS]), "u": u_all}
        for i in range(N_CORES)
    ]
    res = run_bass_kernel_spmd(nc, in_maps, core_ids=list(range(N_CORES)), trace=trace)

    vt = np.concatenate([r["v"] for r in res.results], axis=1)  # [T, B*2]
    out = np.ascontiguousarray(vt.reshape(T, B, 2).transpose(1, 0, 2))
    return out, res


def kernel(x_seq: np.ndarray) -> np.ndarray:
    out, _ = _run(x_seq, trace=False)
    return out



# revision 2
# speedup vs baseline: 1.2961x; 1.2961x over previous
"""Trainium2 Bass kernel for the batched 2D Kalman filter (nn_KalmanFilterWrapper).

Math
----
The reference runs, per trajectory, a Kalman filter over T=4096 steps with a
constant-velocity model.  The gain/covariance recursion (Riccati) is
data-independent, so the scan collapses to a linear time-varying recurrence
with coefficients shared across the whole batch; the 4-state filter decouples
into two identical 2-state scalar filters (one per coordinate), giving
B*2 = 8192 independent scalar sequences.

Blocking time into chunks of C=126 steps turns the filter into, per block,

    out_b = Uz_b @ z_b  +  Uc_b @ s_{b-1}

realized as two PSUM-accumulating matmuls: a [126x128] z-coefficient matmul
plus a [2x128] carry matmul whose rhs is rows 0:2 of the PREVIOUS block's
evicted output tile (the eviction to SBUF produces the carry for free — no
separate carry copy).  Output rows: 0 = p_last, 1 = v_last, 2+j = p_j; rows
0:2 are the filter state that block b+1 consumes.  All coefficients are
precomputed on the host in float64.

Everything on the wire is float16: the rel-err budget (2e-2) dwarfs the
~3.6e-4 this costs, it halves HBM traffic (the kernel is DMA-bound), and
fp16 matmuls run the PE at full rate (1 cycle/row vs 4 for fp32).

Sharding: data-parallel across 8 NeuronCores, 512 trajectories (1024 scalar
sequences) per core.  Layout on device is [time, sequence]; the host
transposes in/out of the reference's [batch, time, 2] layout.
"""

import numpy as np

import concourse.bass as bass
import concourse.bacc as bacc
import concourse.mybir as mybir
from concourse.bass_utils import run_bass_kernel_spmd
from concourse.tile import TileContext

# Problem constants (hardcoded per harness contract).
B = 4096
T = 4096
DT = 1.0
PROCESS_VARIANCE = 1e-05
MEASUREMENT_VARIANCE = 0.1
INIT_ERROR = 1.0

N_CORES = 8
NCOLS = (B * 2) // N_CORES  # 1024 scalar sequences per core
MAIN_C = 126                # steps per full block
LAST_C = T - (T // MAIN_C) * MAIN_C  # 64
NBLK = T // MAIN_C + (1 if LAST_C else 0)  # 33
CHUNK = 512                 # matmul moving free-dim (one fp32 PSUM bank)

DT_F16 = mybir.dt.float16
DT_F32 = mybir.dt.float32


def _riccati():
    F = np.array([[1.0, DT], [0.0, 1.0]], dtype=np.float64)
    I2 = np.eye(2, dtype=np.float64)
    P = INIT_ERROR * I2.copy()
    A = np.zeros((T, 2, 2), dtype=np.float64)
    k = np.zeros((T, 2), dtype=np.float64)
    for t in range(T):
        Pp = F @ P @ F.T + PROCESS_VARIANCE * I2
        s = Pp[0, 0] + MEASUREMENT_VARIANCE
        kt = Pp[:, 0] / s
        k[t] = kt
        KH = np.zeros((2, 2), dtype=np.float64)
        KH[:, 0] = kt
        P = (I2 - KH) @ Pp
        A[t] = (I2 - KH) @ F
    return A, k


def _precompute_u():
    """Host-side Riccati + per-block coefficient matrices, float64 -> f16.

    Returns (Uz [128, NBLK*128], Uc [2, NBLK*128]) in lhsT layout
    [contract, out]: out[m] = sum_j Uz[j, m] z[j] + sum_i Uc[i, m] s_prev[i].
    """
    A, k = _riccati()
    Uz = np.zeros((128, NBLK * 128), dtype=np.float64)
    Uc = np.zeros((2, NBLK * 128), dtype=np.float64)
    for b in range(NBLK):
        t0 = b * MAIN_C
        Cb = MAIN_C if b < NBLK - 1 else LAST_C
        Rc = np.zeros((2, 128), dtype=np.float64)
        Mc = np.eye(2, dtype=np.float64)
        if b == 0:
            Rc[0, 0] = 1.0  # x_{-1} = [z_0, 0]
        uz = Uz[:, b * 128:(b + 1) * 128]
        uc = Uc[:, b * 128:(b + 1) * 128]
        for j in range(Cb):
            t = t0 + j
            Mc = A[t] @ Mc
            Rc = A[t] @ Rc
            Rc[:, j] += k[t]
            uz[:, 2 + j] = Rc[0, :]
            uc[:, 2 + j] = Mc[0, :]
        uz[:, 0] = Rc[0, :]
        uz[:, 1] = Rc[1, :]
        uc[:, 0] = Mc[0, :]
        uc[:, 1] = Mc[1, :]
        if b == 0:
            uc[:, :] = 0.0  # block 0 has no carry matmul
    return (np.ascontiguousarray(Uz.astype(np.float16)),
            np.ascontiguousarray(Uc.astype(np.float16)))


def _build_nc():
    nchunks = NCOLS // CHUNK

    nc = bacc.Bacc()
    z = nc.dram_tensor("z", [T, NCOLS], DT_F16, kind="ExternalInput")
    uz = nc.dram_tensor("uz", [128, NBLK * 128], DT_F16, kind="ExternalInput")
    uc = nc.dram_tensor("uc", [2, NBLK * 128], DT_F16, kind="ExternalInput")
    v = nc.dram_tensor("v", [T, NCOLS], DT_F16, kind="ExternalOutput")

    def cdim(b):
        return MAIN_C if b < NBLK - 1 else LAST_C

    with TileContext(nc) as tc:
        with (
            tc.tile_pool(name="consts", bufs=1) as cpool,
            tc.tile_pool(name="zpool", bufs=6) as zpool,
            tc.tile_pool(name="vpool", bufs=5) as vpool,
            tc.tile_pool(name="psum", bufs=6, space="PSUM") as ppool,
        ):
            uz_t = cpool.tile([128, NBLK * 128], DT_F16)
            nc.sync.dma_start(uz_t[:, :], uz[:, :])
            uc_t = cpool.tile([2, NBLK * 128], DT_F16)
            nc.sync.dma_start(uc_t[:, :], uc[:, :])

            ztiles = {}
            for b in range(NBLK):
                t0, Cb = b * MAIN_C, cdim(b)
                zp = zpool.tile([128, NCOLS], DT_F16, tag="zp")
                nc.sync.dma_start(zp[0:Cb, :], z[t0:t0 + Cb, :])
                ztiles[b] = zp

            vprev = None
            for b in range(NBLK):
                t0, Cb = b * MAIN_C, cdim(b)
                zp = ztiles[b]
                vout = vpool.tile([128, NCOLS], DT_F16, tag="vout")
                for ci in range(nchunks):
                    cols = bass.ds(ci * CHUNK, CHUNK)
                    ps = ppool.tile([128, CHUNK], DT_F32)
                    nc.tensor.matmul(
                        ps[:, :],
                        uz_t[0:Cb, bass.ds(b * 128, 128)],
                        zp[0:Cb, cols],
                        start=True,
                        stop=(b == 0),
                    )
                    if b > 0:
                        nc.tensor.matmul(
                            ps[:, :],
                            uc_t[0:2, bass.ds(b * 128, 128)],
                            vprev[0:2, cols],
                            start=False,
                            stop=True,
                        )
                    # split evictions across ACT and DVE
                    if ci % 2 == 0:
                        nc.scalar.copy(vout[:, cols], ps[:, :])
                    else:
                        nc.vector.tensor_copy(vout[:, cols], ps[:, :])
                nc.sync.dma_start(v[t0:t0 + Cb, :], vout[2:2 + Cb, :])
                vprev = vout
    nc.finalize()
    return nc


_CACHE = {}


def _run(x_seq: np.ndarray, trace: bool = False):
    if "nc" not in _CACHE:
        _CACHE["nc"] = _build_nc()
        _CACHE["u"] = _precompute_u()
    nc = _CACHE["nc"]
    uz, uc = _CACHE["u"]

    x = np.asarray(x_seq)
    assert x.shape == (B, T, 2), x.shape

    # [B, T, 2] -> [T, B*2] fp16; column n = 2*b + c
    zt = np.ascontiguousarray(
        x.transpose(1, 0, 2).reshape(T, B * 2).astype(np.float16))

    in_maps = [
        {"z": np.ascontiguousarray(zt[:, i * NCOLS:(i + 1) * NCOLS]),
         "uz": uz, "uc": uc}
        for i in range(N_CORES)
    ]
    res = run_bass_kernel_spmd(nc, in_maps, core_ids=list(range(N_CORES)), trace=trace)

    vt = np.concatenate([r["v"] for r in res.results], axis=1)  # [T, B*2] f16
    out = np.ascontiguousarray(
        vt.astype(np.float32).reshape(T, B, 2).transpose(1, 0, 2))
    return out, res


def kernel(x_seq: np.ndarray) -> np.ndarray:
    out, _ = _run(x_seq, trace=False)
    return out


# revision 3
# speedup vs baseline: 1.6156x; 1.2465x over previous
"""Trainium2 Bass kernel for the batched 2D Kalman filter (nn_KalmanFilterWrapper).

Math
----
The reference runs, per trajectory, a Kalman filter over T=4096 steps with a
constant-velocity model.  The gain/covariance recursion (Riccati) is
data-independent, so the scan collapses to a linear time-varying recurrence
with coefficients shared across the whole batch; the 4-state filter decouples
into two identical 2-state scalar filters (one per coordinate), giving
B*2 = 8192 independent scalar sequences.

Blocking time into chunks of C=126 steps turns the filter into one
[128x128] @ [128x512] matmul per block and PSUM chunk: contract rows 0,1
carry the filter state from the previous block (p_prev, v_prev), rows 2+j
the block's measurements; output rows 0,1 duplicate the end-of-block state
(next block's carry), rows 2+j the filtered positions.  Block 0 folds the
x0 = [z_0, 0] init into its coefficients (no carry rows, contract 126); the
final short block uses contract 66 (2 carry + 64 measurements).

Engine assignment keeps the serial carry chain short: the PE matmul's state
rows are copied into the next block's z tile by ACT ([2 x 512] per chunk),
while ALL full-tile PSUM->SBUF evictions run on DVE, so the chain never
queues behind an eviction.  Coefficients are precomputed on the host in
float64.

Everything on the wire is float16: the rel-err budget (2e-2) dwarfs the
~3.6e-4 this costs, it halves HBM traffic (the kernel is DMA-bound), and
fp16 matmuls run the PE at full rate.

Sharding: data-parallel across 8 NeuronCores, 512 trajectories (1024 scalar
sequences) per core.  Layout on device is [time, sequence]; the host
transposes in/out of the reference's [batch, time, 2] layout.
"""

import numpy as np

import concourse.bass as bass
import concourse.bacc as bacc
import concourse.mybir as mybir
from concourse.bass_utils import run_bass_kernel_spmd
from concourse.tile import TileContext

# Problem constants (hardcoded per harness contract).
B = 4096
T = 4096
DT = 1.0
PROCESS_VARIANCE = 1e-05
MEASUREMENT_VARIANCE = 0.1
INIT_ERROR = 1.0

N_CORES = 8
NCOLS = (B * 2) // N_CORES  # 1024 scalar sequences per core
MAIN_C = 126                # steps per full block
LAST_C = T - (T // MAIN_C) * MAIN_C  # 64
NBLK = T // MAIN_C + (1 if LAST_C else 0)  # 33
CHUNK = 512                 # matmul moving free-dim (one fp32 PSUM bank)

DT_F16 = mybir.dt.float16
DT_F32 = mybir.dt.float32


def _cdim(b):
    return MAIN_C if b < NBLK - 1 else LAST_C


def _riccati():
    F = np.array([[1.0, DT], [0.0, 1.0]], dtype=np.float64)
    I2 = np.eye(2, dtype=np.float64)
    P = INIT_ERROR * I2.copy()
    A = np.zeros((T, 2, 2), dtype=np.float64)
    k = np.zeros((T, 2), dtype=np.float64)
    for t in range(T):
        Pp = F @ P @ F.T + PROCESS_VARIANCE * I2
        s = Pp[0, 0] + MEASUREMENT_VARIANCE
        kt = Pp[:, 0] / s
        k[t] = kt
        KH = np.zeros((2, 2), dtype=np.float64)
        KH[:, 0] = kt
        P = (I2 - KH) @ Pp
        A[t] = (I2 - KH) @ F
    return A, k


def _precompute_u():
    """Host-side Riccati + per-block lhsT coefficients, float64 -> f16.

    Returns U [128, NBLK*128]; block b's lhsT is cols [b*128, (b+1)*128):
    U[i, m] = coefficient of contract input i in output m.  Contract rows
    0,1 = carry (b>=1), zoff+j = z_j; out cols 0 = p_last, 1 = v_last,
    2+j = p_j.
    """
    A, k = _riccati()
    U = np.zeros((128, NBLK * 128), dtype=np.float64)
    for b in range(NBLK):
        t0 = b * MAIN_C
        L = U[:, b * 128:(b + 1) * 128]
        Rc = np.zeros((2, 128), dtype=np.float64)
        if b == 0:
            Rc[0, 0] = 1.0  # x_{-1} = [z_0, 0]; z_0 is contract input 0
            zoff = 0
        else:
            Rc[0, 0] = 1.0  # carry row 0 = p_prev
            Rc[1, 1] = 1.0  # carry row 1 = v_prev
            zoff = 2
        for j in range(_cdim(b)):
            t = t0 + j
            Rc = A[t] @ Rc
            Rc[:, zoff + j] += k[t]
            L[:, 2 + j] = Rc[0, :]
        L[:, 0] = Rc[0, :]  # p_last (dup) -> next block carry row 0
        L[:, 1] = Rc[1, :]  # v_last      -> next block carry row 1
    return np.ascontiguousarray(U.astype(np.float16))


def _build_nc():
    nchunks = NCOLS // CHUNK

    nc = bacc.Bacc()
    z = nc.dram_tensor("z", [T, NCOLS], DT_F16, kind="ExternalInput")
    u = nc.dram_tensor("u", [128, NBLK * 128], DT_F16, kind="ExternalInput")
    v = nc.dram_tensor("v", [T, NCOLS], DT_F16, kind="ExternalOutput")

    with TileContext(nc) as tc:
        with (
            tc.tile_pool(name="consts", bufs=1) as cpool,
            tc.tile_pool(name="zpool", bufs=6) as zpool,
            tc.tile_pool(name="vpool", bufs=5) as vpool,
            tc.tile_pool(name="psum", bufs=6, space="PSUM") as ppool,
        ):
            u_t = cpool.tile([128, NBLK * 128], DT_F16)
            nc.sync.dma_start(u_t[:, :], u[:, :])

            ztiles = {}
            for b in range(NBLK):
                t0, Cb = b * MAIN_C, _cdim(b)
                zp = zpool.tile([128, NCOLS], DT_F16, tag="zp")
                if b == 0:
                    nc.sync.dma_start(zp[0:Cb, :], z[t0:t0 + Cb, :])
                else:
                    nc.sync.dma_start(zp[2:2 + Cb, :], z[t0:t0 + Cb, :])
                ztiles[b] = zp

            for b in range(NBLK):
                t0, Cb = b * MAIN_C, _cdim(b)
                contract = Cb if b == 0 else Cb + 2
                zp = ztiles[b]
                vout = vpool.tile([128, NCOLS], DT_F16, tag="vout")
                pss = []
                for ci in range(nchunks):
                    cols = bass.ds(ci * CHUNK, CHUNK)
                    ps = ppool.tile([128, CHUNK], DT_F32)
                    nc.tensor.matmul(
                        ps[:, :],
                        u_t[0:contract, bass.ds(b * 128, 128)],
                        zp[0:contract, cols],
                        start=True,
                        stop=True,
                    )
                    # state rows -> next block's carry, on ACT (off the
                    # eviction path so the serial chain stays short)
                    if b + 1 < NBLK:
                        nc.scalar.copy(ztiles[b + 1][0:2, cols], ps[0:2, :])
                    pss.append((ps, cols))
                # full-tile evictions all on DVE
                for ps, cols in pss:
                    nc.vector.tensor_copy(vout[:, cols], ps[:, :])
                nc.sync.dma_start(v[t0:t0 + Cb, :], vout[2:2 + Cb, :])
    nc.finalize()
    return nc


_CACHE = {}


def _run(x_seq: np.ndarray, trace: bool = False):
    if "nc" not in _CACHE:
        _CACHE["nc"] = _build_nc()
        _CACHE["u"] = _precompute_u()
    nc = _CACHE["nc"]
    u_all = _CACHE["u"]

    x = np.asarray(x_seq)
    assert x.shape == (B, T, 2), x.shape

    # [B, T, 2] -> [T, B*2] fp16; column n = 2*b + c
    zt = np.ascontiguousarray(
        x.transpose(1, 0, 2).reshape(T, B * 2).astype(np.float16))

    in_maps = [
        {"z": np.ascontiguousarray(zt[:, i * NCOLS:(i + 1) * NCOLS]), "u": u_all}
        for i in range(N_CORES)
    ]
    res = run_bass_kernel_spmd(nc, in_maps, core_ids=list(range(N_CORES)), trace=trace)

    vt = np.concatenate([r["v"] for r in res.results], axis=1)  # [T, B*2] f16
    out = np.ascontiguousarray(
        vt.astype(np.float32).reshape(T, B, 2).transpose(1, 0, 2))
    return out, res


def kernel(x_seq: np.ndarray) -> np.ndarray:
    out, _ = _run(x_seq, trace=False)
    return out


# revision 6
# speedup vs baseline: 1.6212x; 1.0035x over previous
"""Trainium2 Bass kernel for the batched 2D Kalman filter (nn_KalmanFilterWrapper).

Math
----
The reference runs, per trajectory, a Kalman filter over T=4096 steps with a
constant-velocity model.  The gain/covariance recursion (Riccati) is
data-independent, so the scan collapses to a linear time-varying recurrence
with coefficients shared across the whole batch; the 4-state filter decouples
into two identical 2-state scalar filters (one per coordinate), giving
B*2 = 8192 independent scalar sequences.

Blocking time into chunks of C=126 steps turns the filter into one
[128x128] @ [128x512] matmul per block and PSUM chunk: contract rows 0,1
carry the filter state from the previous block (p_prev, v_prev), rows 2+j
the block's measurements; output rows 0,1 duplicate the end-of-block state
(next block's carry), rows 2+j the filtered positions.  Block 0 folds the
x0 = [z_0, 0] init into its coefficients (no carry rows, contract 126); the
final short block uses contract 66 (2 carry + 64 measurements).

Engine assignment keeps the serial carry chain short: the PE matmul's state
rows are copied into the next block's z tile by ACT ([2 x 512] per chunk),
while ALL full-tile PSUM->SBUF evictions run on DVE, so the chain never
queues behind an eviction.  Coefficients are precomputed on the host in
float64.

Everything on the wire is float16: the rel-err budget (2e-2) dwarfs the
~3.6e-4 this costs, it halves HBM traffic (the kernel is DMA-bound), and
fp16 matmuls run the PE at full rate.

Sharding: data-parallel across 8 NeuronCores, 512 trajectories (1024 scalar
sequences) per core.  Layout on device is [time, sequence]; the host
transposes in/out of the reference's [batch, time, 2] layout.
"""

import numpy as np

import concourse.bass as bass
import concourse.bacc as bacc
import concourse.mybir as mybir
from concourse.bass_utils import run_bass_kernel_spmd
from concourse.tile import TileContext

# Problem constants (hardcoded per harness contract).
B = 4096
T = 4096
DT = 1.0
PROCESS_VARIANCE = 1e-05
MEASUREMENT_VARIANCE = 0.1
INIT_ERROR = 1.0

N_CORES = 8
NCOLS = (B * 2) // N_CORES  # 1024 scalar sequences per core
MAIN_C = 126                # steps per full block
LAST_C = T - (T // MAIN_C) * MAIN_C  # 64
NBLK = T // MAIN_C + (1 if LAST_C else 0)  # 33
CHUNK = 512                 # matmul moving free-dim (one fp32 PSUM bank)

DT_F16 = mybir.dt.float16
DT_F32 = mybir.dt.float32


def _cdim(b):
    return MAIN_C if b < NBLK - 1 else LAST_C


def _riccati():
    F = np.array([[1.0, DT], [0.0, 1.0]], dtype=np.float64)
    I2 = np.eye(2, dtype=np.float64)
    P = INIT_ERROR * I2.copy()
    A = np.zeros((T, 2, 2), dtype=np.float64)
    k = np.zeros((T, 2), dtype=np.float64)
    for t in range(T):
        Pp = F @ P @ F.T + PROCESS_VARIANCE * I2
        s = Pp[0, 0] + MEASUREMENT_VARIANCE
        kt = Pp[:, 0] / s
        k[t] = kt
        KH = np.zeros((2, 2), dtype=np.float64)
        KH[:, 0] = kt
        P = (I2 - KH) @ Pp
        A[t] = (I2 - KH) @ F
    return A, k


def _precompute_u():
    """Host-side Riccati + lhsT coefficients, float64 -> f16.

    The Riccati recursion converges to its steady state well inside block 0
    (fp16-exactly by t=50), so every block b >= 1 shares ONE coefficient
    matrix; the short last block is that same matrix truncated to its first
    66 contract rows (a causal filter has zero coefficients on future
    inputs).  Returns U [128, 256]: cols 0:128 = block-0 lhsT (init folded,
    contract rows j = z_j), cols 128:256 = steady lhsT (contract rows 0,1 =
    carry, 2+j = z_j).  Out cols: 0 = p_last, 1 = v_last, 2+j = p_j.
    """
    A, k = _riccati()
    U = np.zeros((128, 256), dtype=np.float64)
    for sl, t0 in ((0, 0), (1, MAIN_C)):
        L = U[:, sl * 128:(sl + 1) * 128]
        Rc = np.zeros((2, 128), dtype=np.float64)
        if sl == 0:
            Rc[0, 0] = 1.0  # x_{-1} = [z_0, 0]; z_0 is contract input 0
            zoff = 0
        else:
            Rc[0, 0] = 1.0  # carry row 0 = p_prev
            Rc[1, 1] = 1.0  # carry row 1 = v_prev
            zoff = 2
        for j in range(MAIN_C):
            t = t0 + j
            Rc = A[t] @ Rc
            Rc[:, zoff + j] += k[t]
            L[:, 2 + j] = Rc[0, :]
        L[:, 0] = Rc[0, :]  # p_last (dup) -> next block carry row 0
        L[:, 1] = Rc[1, :]  # v_last      -> next block carry row 1
    return np.ascontiguousarray(U.astype(np.float16))


PREFETCH = 12  # z tiles in flight (zpool bufs)


def _build_nc():
    nchunks = NCOLS // CHUNK

    nc = bacc.Bacc()
    z = nc.dram_tensor("z", [T, NCOLS], DT_F16, kind="ExternalInput")
    u = nc.dram_tensor("u", [128, 256], DT_F16, kind="ExternalInput")
    v = nc.dram_tensor("v", [T, NCOLS], DT_F16, kind="ExternalOutput")

    with TileContext(nc) as tc:
        with (
            tc.tile_pool(name="consts", bufs=1) as cpool,
            tc.tile_pool(name="zpool", bufs=PREFETCH) as zpool,
            tc.tile_pool(name="vpool", bufs=6) as vpool,
            tc.tile_pool(name="psum", bufs=8, space="PSUM") as ppool,
        ):
            ztiles = {}

            def fetch_z(b, eng=None):
                t0, Cb = b * MAIN_C, _cdim(b)
                zp = zpool.tile([128, NCOLS], DT_F16, tag="zp")
                if b == 0:
                    nc.sync.dma_start(zp[0:Cb, :], z[t0:t0 + Cb, :])
                else:
                    (eng or nc.scalar).dma_start(zp[2:2 + Cb, :], z[t0:t0 + Cb, :])
                ztiles[b] = zp

            # block 0's measurements gate the whole chain: trigger first
            fetch_z(0)
            u_t = cpool.tile([128, 256], DT_F16)
            nc.sync.dma_start(u_t[:, :], u[:, :])
            for b in range(1, PREFETCH):
                fetch_z(b)

            for b in range(NBLK):
                t0, Cb = b * MAIN_C, _cdim(b)
                contract = Cb if b == 0 else Cb + 2
                usel = bass.ds(0, 128) if b == 0 else bass.ds(128, 128)
                zp = ztiles.pop(b)
                vout = vpool.tile([128, NCOLS], DT_F16, tag="vout")
                pss = []
                for ci in range(nchunks):
                    cols = bass.ds(ci * CHUNK, CHUNK)
                    ps = ppool.tile([128, CHUNK], DT_F32)
                    nc.tensor.matmul(
                        ps[:, :],
                        u_t[0:contract, usel],
                        zp[0:contract, cols],
                        start=True,
                        stop=True,
                    )
                    # state rows -> next block's carry, on ACT (off the
                    # eviction path so the serial chain stays short)
                    if b + 1 < NBLK:
                        nc.scalar.copy(ztiles[b + 1][0:2, cols], ps[0:2, :])
                    pss.append((ps, cols))
                # full-tile evictions all on DVE
                for ps, cols in pss:
                    nc.vector.tensor_copy(vout[:, cols], ps[:, :])
                # out-DMA trigger BEFORE the next z prefetch so it never
                # queues behind a buffer-throttled input trigger on Sync
                nc.sync.dma_start(v[t0:t0 + Cb, :], vout[2:2 + Cb, :])
                if b + PREFETCH < NBLK:
                    fetch_z(b + PREFETCH)
    nc.finalize()
    return nc


_CACHE = {}


def _run(x_seq: np.ndarray, trace: bool = False):
    if "nc" not in _CACHE:
        _CACHE["nc"] = _build_nc()
        _CACHE["u"] = _precompute_u()
    nc = _CACHE["nc"]
    u_all = _CACHE["u"]

    x = np.asarray(x_seq)
    assert x.shape == (B, T, 2), x.shape

    # [B, T, 2] -> [T, B*2] fp16; column n = 2*b + c
    zt = np.ascontiguousarray(
        x.transpose(1, 0, 2).reshape(T, B * 2).astype(np.float16))

    in_maps = [
        {"z": np.ascontiguousarray(zt[:, i * NCOLS:(i + 1) * NCOLS]), "u": u_all}
        for i in range(N_CORES)
    ]
    res = run_bass_kernel_spmd(nc, in_maps, core_ids=list(range(N_CORES)), trace=trace)

    vt = np.concatenate([r["v"] for r in res.results], axis=1)  # [T, B*2] f16
    out = np.ascontiguousarray(
        vt.astype(np.float32).reshape(T, B, 2).transpose(1, 0, 2))
    return out, res


def kernel(x_seq: np.ndarray) -> np.ndarray:
    out, _ = _run(x_seq, trace=False)
    return out


# revision 7
# speedup vs baseline: 1.7820x; 1.0992x over previous
"""Trainium2 Bass kernel for the batched 2D Kalman filter (nn_KalmanFilterWrapper).

Math
----
The reference runs, per trajectory, a Kalman filter over T=4096 steps with a
constant-velocity model.  The gain/covariance recursion (Riccati) is
data-independent, so the scan collapses to a linear time-varying recurrence
with coefficients shared across the whole batch; the 4-state filter decouples
into two identical 2-state scalar filters (one per coordinate), giving
B*2 = 8192 independent scalar sequences.

Blocking time into chunks of C=126 steps turns the filter into one
[128x128] @ [128x512] matmul per block and PSUM chunk: contract rows 0,1
carry the filter state from the previous block (p_prev, v_prev), rows 2+j
the block's measurements; output rows 0,1 duplicate the end-of-block state
(next block's carry), rows 2+j the filtered positions.  Block 0 folds the
x0 = [z_0, 0] init into its coefficients (no carry rows, contract 126); the
final short block uses contract 66 (2 carry + 64 measurements).

Engine assignment keeps the serial carry chain short: the PE matmul's state
rows are copied into the next block's z tile by ACT ([2 x 512] per chunk),
while ALL full-tile PSUM->SBUF evictions run on DVE, so the chain never
queues behind an eviction.  Coefficients are precomputed on the host in
float64.

Everything on the wire is float16: the rel-err budget (2e-2) dwarfs the
~3.6e-4 this costs, it halves HBM traffic (the kernel is DMA-bound), and
fp16 matmuls run the PE at full rate.

Sharding: data-parallel across 8 NeuronCores, 512 trajectories (1024 scalar
sequences) per core.  Layout on device is [time, sequence]; the host
transposes in/out of the reference's [batch, time, 2] layout.
"""

import numpy as np

import concourse.bass as bass
import concourse.bacc as bacc
import concourse.mybir as mybir
from concourse.bass_utils import run_bass_kernel_spmd
from concourse.tile import TileContext

# Problem constants (hardcoded per harness contract).
B = 4096
T = 4096
DT = 1.0
PROCESS_VARIANCE = 1e-05
MEASUREMENT_VARIANCE = 0.1
INIT_ERROR = 1.0

N_CORES = 8
NCOLS = (B * 2) // N_CORES  # 1024 scalar sequences per core
MAIN_C = 126                # steps per full block
LAST_C = T - (T // MAIN_C) * MAIN_C  # 64
NBLK = T // MAIN_C + (1 if LAST_C else 0)  # 33
CHUNK = 512                 # matmul moving free-dim (one fp32 PSUM bank)

DT_F16 = mybir.dt.float16
DT_F32 = mybir.dt.float32


def _cdim(b):
    return MAIN_C if b < NBLK - 1 else LAST_C


def _riccati():
    F = np.array([[1.0, DT], [0.0, 1.0]], dtype=np.float64)
    I2 = np.eye(2, dtype=np.float64)
    P = INIT_ERROR * I2.copy()
    A = np.zeros((T, 2, 2), dtype=np.float64)
    k = np.zeros((T, 2), dtype=np.float64)
    for t in range(T):
        Pp = F @ P @ F.T + PROCESS_VARIANCE * I2
        s = Pp[0, 0] + MEASUREMENT_VARIANCE
        kt = Pp[:, 0] / s
        k[t] = kt
        KH = np.zeros((2, 2), dtype=np.float64)
        KH[:, 0] = kt
        P = (I2 - KH) @ Pp
        A[t] = (I2 - KH) @ F
    return A, k


def _precompute_u():
    """Host-side Riccati + lhsT coefficients, float64 -> f16.

    The Riccati recursion converges to its steady state well inside block 0
    (fp16-exactly by t=50), so every block b >= 1 shares ONE coefficient
    matrix; the short last block is that same matrix truncated to its first
    66 contract rows (a causal filter has zero coefficients on future
    inputs).  Returns U [128, 256]: cols 0:128 = block-0 lhsT (init folded,
    contract rows j = z_j), cols 128:256 = steady lhsT (contract rows 0,1 =
    carry, 2+j = z_j).  Out cols: 0 = p_last, 1 = v_last, 2+j = p_j.
    """
    A, k = _riccati()
    U = np.zeros((128, 256), dtype=np.float64)
    for sl, t0 in ((0, 0), (1, MAIN_C)):
        L = U[:, sl * 128:(sl + 1) * 128]
        Rc = np.zeros((2, 128), dtype=np.float64)
        if sl == 0:
            Rc[0, 0] = 1.0  # x_{-1} = [z_0, 0]; z_0 is contract input 0
            zoff = 0
        else:
            Rc[0, 0] = 1.0  # carry row 0 = p_prev
            Rc[1, 1] = 1.0  # carry row 1 = v_prev
            zoff = 2
        for j in range(MAIN_C):
            t = t0 + j
            Rc = A[t] @ Rc
            Rc[:, zoff + j] += k[t]
            L[:, 2 + j] = Rc[0, :]
        L[:, 0] = Rc[0, :]  # p_last (dup) -> next block carry row 0
        L[:, 1] = Rc[1, :]  # v_last      -> next block carry row 1
    return np.ascontiguousarray(U.astype(np.float16))


PREFETCH = 12  # z tiles in flight (zpool bufs)


def _build_nc():
    nchunks = NCOLS // CHUNK

    nc = bacc.Bacc()
    z = nc.dram_tensor("z", [T, NCOLS], DT_F16, kind="ExternalInput")
    u = nc.dram_tensor("u", [128, 256], DT_F16, kind="ExternalInput")
    v = nc.dram_tensor("v", [T, NCOLS], DT_F16, kind="ExternalOutput")

    with TileContext(nc) as tc:
        with (
            tc.tile_pool(name="consts", bufs=1) as cpool,
            tc.tile_pool(name="zpool", bufs=PREFETCH) as zpool,
            tc.tile_pool(name="vpool", bufs=6) as vpool,
            tc.tile_pool(name="psum", bufs=8, space="PSUM") as ppool,
        ):
            ztiles = {}

            def fetch_z(b, eng=None):
                t0, Cb = b * MAIN_C, _cdim(b)
                zp = zpool.tile([128, NCOLS], DT_F16, tag="zp")
                if b == 0:
                    nc.sync.dma_start(zp[0:Cb, :], z[t0:t0 + Cb, :])
                else:
                    nc.sync.dma_start(zp[2:2 + Cb, :], z[t0:t0 + Cb, :])
                ztiles[b] = zp

            # block 0's measurements gate the whole chain: trigger first
            fetch_z(0)
            u_t = cpool.tile([128, 256], DT_F16)
            nc.sync.dma_start(u_t[:, :], u[:, :])
            for b in range(1, PREFETCH):
                fetch_z(b)

            for b in range(NBLK):
                t0, Cb = b * MAIN_C, _cdim(b)
                contract = Cb if b == 0 else Cb + 2
                usel = bass.ds(0, 128) if b == 0 else bass.ds(128, 128)
                zp = ztiles.pop(b)
                vout = vpool.tile([128, NCOLS], DT_F16, tag="vout")
                pss = []
                for ci in range(nchunks):
                    cols = bass.ds(ci * CHUNK, CHUNK)
                    ps = ppool.tile([128, CHUNK], DT_F32)
                    nc.tensor.matmul(
                        ps[:, :],
                        u_t[0:contract, usel],
                        zp[0:contract, cols],
                        start=True,
                        stop=True,
                    )
                    # state rows -> next block's carry, on ACT (off the
                    # eviction path so the serial chain stays short)
                    if b + 1 < NBLK:
                        nc.scalar.copy(ztiles[b + 1][0:2, cols], ps[0:2, :])
                    pss.append((ps, cols))
                # full-tile evictions all on DVE
                for ps, cols in pss:
                    nc.vector.tensor_copy(vout[:, cols], ps[:, :])
                # out-DMA trigger BEFORE the next z prefetch so it never
                # queues behind a buffer-throttled input trigger on Sync
                nc.sync.dma_start(v[t0:t0 + Cb, :], vout[2:2 + Cb, :])
                if b + PREFETCH < NBLK:
                    fetch_z(b + PREFETCH)
    nc.finalize()
    return nc


_CACHE = {}


def _run(x_seq: np.ndarray, trace: bool = False):
    if "nc" not in _CACHE:
        _CACHE["nc"] = _build_nc()
        _CACHE["u"] = _precompute_u()
    nc = _CACHE["nc"]
    u_all = _CACHE["u"]

    x = np.asarray(x_seq)
    assert x.shape == (B, T, 2), x.shape

    # [B, T, 2] -> [T, B*2] fp16; column n = 2*b + c
    zt = np.ascontiguousarray(
        x.transpose(1, 0, 2).reshape(T, B * 2).astype(np.float16))

    in_maps = [
        {"z": np.ascontiguousarray(zt[:, i * NCOLS:(i + 1) * NCOLS]), "u": u_all}
        for i in range(N_CORES)
    ]
    res = run_bass_kernel_spmd(nc, in_maps, core_ids=list(range(N_CORES)), trace=trace)

    vt = np.concatenate([r["v"] for r in res.results], axis=1)  # [T, B*2] f16
    out = np.ascontiguousarray(
        vt.astype(np.float32).reshape(T, B, 2).transpose(1, 0, 2))
    return out, res


def kernel(x_seq: np.ndarray) -> np.ndarray:
    out, _ = _run(x_seq, trace=False)
    return out


# revision 8
# speedup vs baseline: 1.8876x; 1.0592x over previous
"""Trainium2 Bass kernel for the batched 2D Kalman filter (nn_KalmanFilterWrapper).

Math
----
The reference runs, per trajectory, a Kalman filter over T=4096 steps with a
constant-velocity model.  The gain/covariance recursion (Riccati) is
data-independent, so the scan collapses to a linear time-varying recurrence
with coefficients shared across the whole batch; the 4-state filter decouples
into two identical 2-state scalar filters (one per coordinate), giving
B*2 = 8192 independent scalar sequences.

Blocking time into chunks of C=126 steps turns the filter into one
[128x128] @ [128x512] matmul per block and PSUM chunk: contract rows 0,1
carry the filter state from the previous block, rows 2+j the block's
measurements; output rows 0,1 duplicate the end-of-block state (next block's
carry), rows 2+j the filtered positions.  The Riccati recursion reaches
steady state inside block 0, so only two coefficient matrices exist: block 0
(init folded in, carry coefficients zero — the host zero-fills those rows)
and the shared steady-state matrix used by every other block, including the
short zero-padded last one.  Coefficients are precomputed on the host in
float64.

Layout / engines
----------------
Everything on the wire is float16: the rel-err budget (2e-2) dwarfs the
~3.6e-4 this costs, it halves HBM traffic (the kernel is DMA-bound), and
fp16 matmuls run the PE at full rate.

DMA descriptor generation is serial on the issuing sequencer (~7 ns/desc),
so both z and v live in DRAM as [128, NBLK*NCOLS] SLABS: partition row
2+j holds step j of every block side by side.  One DMA then moves a GROUP
of up to 4 consecutive blocks with just 128 descriptors of contiguous
8 KB lines (vs 126 x 2 KB per block in [time, batch] layout), cutting
trigger-side generation ~4x.  Group sizes ramp 1,1,2,4,... so the first
matmul isn't gated on a full group load.  The host packs/unpacks the slabs.

The serial carry chain stays short: ACT copies the matmul's state rows into
the next block's carry slot ([2 x 512] per chunk) while ALL full-tile
PSUM->SBUF evictions run on DVE, so the chain never queues behind an
eviction.

Sharding: data-parallel across 8 NeuronCores, 512 trajectories (1024 scalar
sequences) per core.
"""

import numpy as np

import concourse.bass as bass
import concourse.bacc as bacc
import concourse.mybir as mybir
from concourse.bass_utils import run_bass_kernel_spmd
from concourse.tile import TileContext

# Problem constants (hardcoded per harness contract).
B = 4096
T = 4096
DT = 1.0
PROCESS_VARIANCE = 1e-05
MEASUREMENT_VARIANCE = 0.1
INIT_ERROR = 1.0

N_CORES = 8
NCOLS = (B * 2) // N_CORES  # 1024 scalar sequences per core
MAIN_C = 126                # steps per block
LAST_C = T - (T // MAIN_C) * MAIN_C  # 64 (block 32, zero-padded to 126)
NBLK = T // MAIN_C + (1 if LAST_C else 0)  # 33
CHUNK = 512                 # matmul moving free-dim (one fp32 PSUM bank)

# blocks per DMA group: ramp in for a fast first matmul, 1-block tail
GSIZES = [1, 1, 2] + [4] * 7 + [1]
assert sum(GSIZES) == NBLK
GBASE = np.cumsum([0] + GSIZES).tolist()
GMAX = max(GSIZES)

DT_F16 = mybir.dt.float16
DT_F32 = mybir.dt.float32


def _riccati():
    F = np.array([[1.0, DT], [0.0, 1.0]], dtype=np.float64)
    I2 = np.eye(2, dtype=np.float64)
    P = INIT_ERROR * I2.copy()
    A = np.zeros((T, 2, 2), dtype=np.float64)
    k = np.zeros((T, 2), dtype=np.float64)
    for t in range(T):
        Pp = F @ P @ F.T + PROCESS_VARIANCE * I2
        s = Pp[0, 0] + MEASUREMENT_VARIANCE
        kt = Pp[:, 0] / s
        k[t] = kt
        KH = np.zeros((2, 2), dtype=np.float64)
        KH[:, 0] = kt
        P = (I2 - KH) @ Pp
        A[t] = (I2 - KH) @ F
    return A, k


def _precompute_u():
    """Returns U [128, 256] f16 in lhsT layout (U[i, m] = coefficient of
    contract input i in output m).  Cols 0:128 = block 0 (init folded, carry
    rows zero), cols 128:256 = steady-state block.  Contract rows 0,1 =
    carry, 2+j = z_j; out cols 0 = p_last, 1 = v_last, 2+j = p_j."""
    A, k = _riccati()
    U = np.zeros((128, 256), dtype=np.float64)
    for sl, t0 in ((0, 0), (1, MAIN_C)):
        L = U[:, sl * 128:(sl + 1) * 128]
        Rc = np.zeros((2, 128), dtype=np.float64)
        if sl == 0:
            Rc[0, 2] = 1.0  # x_{-1} = [z_0, 0]; z_0 is contract input 2
        else:
            Rc[0, 0] = 1.0  # carry row 0 = p_prev
            Rc[1, 1] = 1.0  # carry row 1 = v_prev
        for j in range(MAIN_C):
            t = t0 + j
            Rc = A[t] @ Rc
            Rc[:, 2 + j] += k[t]
            L[:, 2 + j] = Rc[0, :]
        L[:, 0] = Rc[0, :]  # p_last (dup) -> next block carry row 0
        L[:, 1] = Rc[1, :]  # v_last      -> next block carry row 1
    return np.ascontiguousarray(U.astype(np.float16))


def _build_nc():
    nchunks = NCOLS // CHUNK
    ngroups = len(GSIZES)

    nc = bacc.Bacc()
    z = nc.dram_tensor("z", [128, NBLK * NCOLS], DT_F16, kind="ExternalInput")
    u = nc.dram_tensor("u", [128, 256], DT_F16, kind="ExternalInput")
    v = nc.dram_tensor("v", [128, NBLK * NCOLS], DT_F16, kind="ExternalOutput")

    with TileContext(nc) as tc:
        with (
            tc.tile_pool(name="consts", bufs=1) as cpool,
            tc.tile_pool(name="zpool", bufs=3) as zpool,
            tc.tile_pool(name="vpool", bufs=3) as vpool,
            tc.tile_pool(name="psum", bufs=8, space="PSUM") as ppool,
        ):
            gtiles = {}

            def fetch_group(g):
                gs = GSIZES[g]
                gt = zpool.tile([128, GMAX * NCOLS], DT_F16, tag="zg")
                nc.sync.dma_start(
                    gt[:, 0:gs * NCOLS],
                    z[:, GBASE[g] * NCOLS:(GBASE[g] + gs) * NCOLS],
                )
                gtiles[g] = gt

            def ztile_of(b):
                """(tile, col offset) holding block b."""
                g = 0
                while GBASE[g + 1] <= b:
                    g += 1
                return gtiles[g], (b - GBASE[g]) * NCOLS

            fetch_group(0)
            u_t = cpool.tile([128, 256], DT_F16)
            nc.sync.dma_start(u_t[:, :], u[:, :])
            fetch_group(1)
            fetch_group(2)

            for g in range(ngroups):
                gs = GSIZES[g]
                vgt = vpool.tile([128, GMAX * NCOLS], DT_F16, tag="vg")
                for q in range(gs):
                    b = GBASE[g] + q
                    zgt = gtiles[g]
                    usel = bass.ds(0, 128) if b == 0 else bass.ds(128, 128)
                    pss = []
                    for ci in range(nchunks):
                        zcols = bass.ds(q * NCOLS + ci * CHUNK, CHUNK)
                        ps = ppool.tile([128, CHUNK], DT_F32)
                        nc.tensor.matmul(
                            ps[:, :], u_t[0:128, usel], zgt[0:128, zcols],
                            start=True, stop=True,
                        )
                        # state rows -> next block's carry slot, on ACT (off
                        # the eviction path so the serial chain stays short)
                        if b + 1 < NBLK:
                            nt, noff = ztile_of(b + 1)
                            nc.scalar.copy(
                                nt[0:2, bass.ds(noff + ci * CHUNK, CHUNK)],
                                ps[0:2, :],
                            )
                        pss.append((ps, zcols))
                    # full-tile evictions all on DVE
                    for ps, cols in pss:
                        nc.vector.tensor_copy(vgt[:, cols], ps[:, :])
                # out-DMA trigger BEFORE the next prefetch so it never queues
                # behind a buffer-throttled input trigger on Sync
                nc.sync.dma_start(
                    v[2:128, GBASE[g] * NCOLS:(GBASE[g] + gs) * NCOLS],
                    vgt[2:128, 0:gs * NCOLS],
                )
                if g + 3 < ngroups:
                    fetch_group(g + 3)
    nc.finalize()
    return nc


_CACHE = {}


def _pack_z(x):
    """[B, T, 2] f32 -> slab [128, NBLK, B*2] f16:
    row 2+j, slab b = measurements at step b*126+j (zero-padded)."""
    zt = x.transpose(1, 0, 2).reshape(T, B * 2).astype(np.float16)
    ztp = np.zeros((NBLK * MAIN_C, B * 2), np.float16)
    ztp[:T] = zt
    slab = np.zeros((128, NBLK, B * 2), np.float16)
    slab[2:128] = ztp.reshape(NBLK, MAIN_C, B * 2).transpose(1, 0, 2)
    return slab


def _unpack_v(v_slab):
    """slab [128, NBLK, B*2] f16 -> [B, T, 2] f32."""
    vt = v_slab[2:128].transpose(1, 0, 2).reshape(NBLK * MAIN_C, B * 2)[:T]
    return np.ascontiguousarray(
        vt.astype(np.float32).reshape(T, B, 2).transpose(1, 0, 2))


def _run(x_seq: np.ndarray, trace: bool = False):
    if "nc" not in _CACHE:
        _CACHE["nc"] = _build_nc()
        _CACHE["u"] = _precompute_u()
    nc = _CACHE["nc"]
    u_all = _CACHE["u"]

    x = np.asarray(x_seq)
    assert x.shape == (B, T, 2), x.shape

    slab = _pack_z(x)
    in_maps = [
        {"z": np.ascontiguousarray(
            slab[:, :, i * NCOLS:(i + 1) * NCOLS]).reshape(128, NBLK * NCOLS),
         "u": u_all}
        for i in range(N_CORES)
    ]
    res = run_bass_kernel_spmd(nc, in_maps, core_ids=list(range(N_CORES)), trace=trace)

    v_slab = np.concatenate(
        [r["v"].reshape(128, NBLK, NCOLS) for r in res.results], axis=2)
    return _unpack_v(v_slab), res


def kernel(x_seq: np.ndarray) -> np.ndarray:
    out, _ = _run(x_seq, trace=False)
    return out


# revision 10
# speedup vs baseline: 1.9009x; 1.0071x over previous
"""Trainium2 Bass kernel for the batched 2D Kalman filter (nn_KalmanFilterWrapper).

Math
----
The reference runs, per trajectory, a Kalman filter over T=4096 steps with a
constant-velocity model.  The gain/covariance recursion (Riccati) is
data-independent, so the scan collapses to a linear time-varying recurrence
with coefficients shared across the whole batch; the 4-state filter decouples
into two identical 2-state scalar filters (one per coordinate), giving
B*2 = 8192 independent scalar sequences.

Blocking time into chunks of C=126 steps turns the filter into one
[128x128] @ [128x512] matmul per block and PSUM chunk: contract rows 0,1
carry the filter state from the previous block, rows 2+j the block's
measurements; output rows 0,1 duplicate the end-of-block state (next block's
carry), rows 2+j the filtered positions.  The Riccati recursion reaches
steady state inside block 0, so only two coefficient matrices exist: block 0
(init folded in, carry coefficients zero — the host zero-fills those rows)
and the shared steady-state matrix used by every other block, including the
short zero-padded last one.  Coefficients are precomputed on the host in
float64.

Layout / engines
----------------
Everything on the wire is float16: the rel-err budget (2e-2) dwarfs the
~3.6e-4 this costs, it halves HBM traffic (the kernel is DMA-bound), and
fp16 matmuls run the PE at full rate.

DMA descriptor generation is serial on the issuing sequencer (~7 ns/desc),
so both z and v live in DRAM as [128, NBLK*NCOLS] SLABS: partition row
2+j holds step j of every block side by side.  One DMA then moves a GROUP
of up to 4 consecutive blocks with just 128 descriptors of contiguous
8 KB lines (vs 126 x 2 KB per block in [time, batch] layout), cutting
trigger-side generation ~4x.  Group sizes ramp 1,1,2,4,... so the first
matmul isn't gated on a full group load.  The host packs/unpacks the slabs.

The serial carry chain stays short: ACT copies the matmul's state rows into
the next block's carry slot ([2 x 512] per chunk) while ALL full-tile
PSUM->SBUF evictions run on DVE, so the chain never queues behind an
eviction.

Sharding: data-parallel across 8 NeuronCores, 512 trajectories (1024 scalar
sequences) per core.
"""

import numpy as np

import concourse.bass as bass
import concourse.bacc as bacc
import concourse.mybir as mybir
from concourse.bass_utils import run_bass_kernel_spmd
from concourse.tile import TileContext

# Problem constants (hardcoded per harness contract).
B = 4096
T = 4096
DT = 1.0
PROCESS_VARIANCE = 1e-05
MEASUREMENT_VARIANCE = 0.1
INIT_ERROR = 1.0

N_CORES = 8
NCOLS = (B * 2) // N_CORES  # 1024 scalar sequences per core
MAIN_C = 126                # steps per block
LAST_C = T - (T // MAIN_C) * MAIN_C  # 64 (block 32, zero-padded to 126)
NBLK = T // MAIN_C + (1 if LAST_C else 0)  # 33
CHUNK = 512                 # matmul moving free-dim (one fp32 PSUM bank)

# blocks per DMA group: ramp in for a fast first matmul, 1-block tail
GSIZES = [1, 1, 2] + [4] * 7 + [1]
assert sum(GSIZES) == NBLK
GBASE = np.cumsum([0] + GSIZES).tolist()
GMAX = max(GSIZES)

DT_F16 = mybir.dt.float16
DT_F32 = mybir.dt.float32


def _riccati():
    F = np.array([[1.0, DT], [0.0, 1.0]], dtype=np.float64)
    I2 = np.eye(2, dtype=np.float64)
    P = INIT_ERROR * I2.copy()
    A = np.zeros((T, 2, 2), dtype=np.float64)
    k = np.zeros((T, 2), dtype=np.float64)
    for t in range(T):
        Pp = F @ P @ F.T + PROCESS_VARIANCE * I2
        s = Pp[0, 0] + MEASUREMENT_VARIANCE
        kt = Pp[:, 0] / s
        k[t] = kt
        KH = np.zeros((2, 2), dtype=np.float64)
        KH[:, 0] = kt
        P = (I2 - KH) @ Pp
        A[t] = (I2 - KH) @ F
    return A, k


def _precompute_u():
    """Returns U [128, 256] f16 in lhsT layout (U[i, m] = coefficient of
    contract input i in output m).  Cols 0:128 = block 0 (init folded, carry
    rows zero), cols 128:256 = steady-state block.  Contract rows 0,1 =
    carry, 2+j = z_j; out cols 0 = p_last, 1 = v_last, 2+j = p_j."""
    A, k = _riccati()
    U = np.zeros((128, 256), dtype=np.float64)
    for sl, t0 in ((0, 0), (1, MAIN_C)):
        L = U[:, sl * 128:(sl + 1) * 128]
        Rc = np.zeros((2, 128), dtype=np.float64)
        if sl == 0:
            Rc[0, 2] = 1.0  # x_{-1} = [z_0, 0]; z_0 is contract input 2
        else:
            Rc[0, 0] = 1.0  # carry row 0 = p_prev
            Rc[1, 1] = 1.0  # carry row 1 = v_prev
        for j in range(MAIN_C):
            t = t0 + j
            Rc = A[t] @ Rc
            Rc[:, 2 + j] += k[t]
            L[:, 2 + j] = Rc[0, :]
        L[:, 0] = Rc[0, :]  # p_last (dup) -> next block carry row 0
        L[:, 1] = Rc[1, :]  # v_last      -> next block carry row 1
    return np.ascontiguousarray(U.astype(np.float16))


def _build_nc():
    nchunks = NCOLS // CHUNK
    ngroups = len(GSIZES)

    nc = bacc.Bacc()
    # group 0's measurements and the coefficients ride one DMA: z0u cols
    # 0:NCOLS = block 0 (carry rows host-zeroed), NCOLS:NCOLS+256 = U
    z0u = nc.dram_tensor("z0u", [128, NCOLS + 256], DT_F16, kind="ExternalInput")
    z = nc.dram_tensor("z", [128, NBLK * NCOLS], DT_F16, kind="ExternalInput")
    v = nc.dram_tensor("v", [128, NBLK * NCOLS], DT_F16, kind="ExternalOutput")

    with TileContext(nc) as tc:
        with (
            tc.tile_pool(name="consts", bufs=1) as cpool,
            tc.tile_pool(name="zpool", bufs=4) as zpool,
            tc.tile_pool(name="vpool", bufs=4) as vpool,
            tc.tile_pool(name="psum", bufs=8, space="PSUM") as ppool,
        ):
            gtiles = {}

            def fetch_group(g):
                # carry rows 0,1 are ACT-written, not loaded (group 0's came
                # from the host inside z0u)
                gs = GSIZES[g]
                gt = zpool.tile([128, GMAX * NCOLS], DT_F16, tag="zg")
                nc.sync.dma_start(
                    gt[2:128, 0:gs * NCOLS],
                    z[2:128, GBASE[g] * NCOLS:(GBASE[g] + gs) * NCOLS],
                )
                gtiles[g] = gt

            def ztile_of(b):
                """(tile, col offset) holding block b."""
                g = 0
                while GBASE[g + 1] <= b:
                    g += 1
                return gtiles[g], (b - GBASE[g]) * NCOLS

            g0u = cpool.tile([128, NCOLS + 256], DT_F16)
            nc.sync.dma_start(g0u[:, :], z0u[:, :])
            gtiles[0] = g0u
            u_t = g0u  # coefficient cols live at NCOLS + [0, 256)
            for g in range(1, 5):
                fetch_group(g)

            for g in range(ngroups):
                gs = GSIZES[g]
                vgt = vpool.tile([128, GMAX * NCOLS], DT_F16, tag="vg")
                gout0 = 0  # first block of the group not yet DMA'd out
                for q in range(gs):
                    b = GBASE[g] + q
                    zgt = gtiles[g]
                    usel = bass.ds(NCOLS, 128) if b == 0 else bass.ds(NCOLS + 128, 128)
                    pss = []
                    for ci in range(nchunks):
                        zcols = bass.ds(q * NCOLS + ci * CHUNK, CHUNK)
                        ps = ppool.tile([128, CHUNK], DT_F32)
                        nc.tensor.matmul(
                            ps[:, :], u_t[0:128, usel], zgt[0:128, zcols],
                            start=True, stop=True,
                        )
                        # state rows -> next block's carry slot, on ACT (off
                        # the eviction path so the serial chain stays short)
                        if b + 1 < NBLK:
                            nt, noff = ztile_of(b + 1)
                            nc.scalar.copy(
                                nt[0:2, bass.ds(noff + ci * CHUNK, CHUNK)],
                                ps[0:2, :],
                            )
                        pss.append((ps, zcols))
                    # full-tile evictions all on DVE
                    for ps, cols in pss:
                        nc.vector.tensor_copy(vgt[:, cols], ps[:, :])
                    # drain finished pairs of blocks early so outputs never
                    # bunch up at the end of the run
                    if q == gs - 1 or q == gout0 + 1:
                        c0, c1 = GBASE[g] + gout0, GBASE[g] + q + 1
                        nc.sync.dma_start(
                            v[2:128, c0 * NCOLS:c1 * NCOLS],
                            vgt[2:128, gout0 * NCOLS:(q + 1) * NCOLS],
                        )
                        gout0 = q + 1
                if g + 5 < ngroups:
                    fetch_group(g + 5)
    nc.finalize()
    return nc


_CACHE = {}


def _pack_z(x):
    """[B, T, 2] f32 -> slab [128, NBLK, B*2] f16:
    row 2+j, slab b = measurements at step b*126+j (zero-padded)."""
    zt = x.transpose(1, 0, 2).reshape(T, B * 2).astype(np.float16)
    ztp = np.zeros((NBLK * MAIN_C, B * 2), np.float16)
    ztp[:T] = zt
    slab = np.zeros((128, NBLK, B * 2), np.float16)
    slab[2:128] = ztp.reshape(NBLK, MAIN_C, B * 2).transpose(1, 0, 2)
    return slab


def _unpack_v(v_slab):
    """slab [128, NBLK, B*2] f16 -> [B, T, 2] f32."""
    vt = v_slab[2:128].transpose(1, 0, 2).reshape(NBLK * MAIN_C, B * 2)[:T]
    return np.ascontiguousarray(
        vt.astype(np.float32).reshape(T, B, 2).transpose(1, 0, 2))


def _run(x_seq: np.ndarray, trace: bool = False):
    if "nc" not in _CACHE:
        _CACHE["nc"] = _build_nc()
        _CACHE["u"] = _precompute_u()
    nc = _CACHE["nc"]
    u_all = _CACHE["u"]

    x = np.asarray(x_seq)
    assert x.shape == (B, T, 2), x.shape

    slab = _pack_z(x)
    in_maps = []
    for i in range(N_CORES):
        zi = np.ascontiguousarray(
            slab[:, :, i * NCOLS:(i + 1) * NCOLS]).reshape(128, NBLK * NCOLS)
        z0u = np.concatenate([zi[:, 0:NCOLS], u_all], axis=1)
        in_maps.append({"z": zi, "z0u": np.ascontiguousarray(z0u)})
    res = run_bass_kernel_spmd(nc, in_maps, core_ids=list(range(N_CORES)), trace=trace)

    v_slab = np.concatenate(
        [r["v"].reshape(128, NBLK, NCOLS) for r in res.results], axis=2)
    return _unpack_v(v_slab), res


def kernel(x_seq: np.ndarray) -> np.ndarray:
    out, _ = _run(x_seq, trace=False)
    return out


# revision 11
# speedup vs baseline: 1.9917x; 1.0478x over previous
"""Trainium2 Bass kernel for the batched 2D Kalman filter (nn_KalmanFilterWrapper).

Math
----
The reference runs, per trajectory, a Kalman filter over T=4096 steps with a
constant-velocity model.  The gain/covariance recursion (Riccati) is
data-independent, so the scan collapses to a linear time-varying recurrence
with coefficients shared across the whole batch; the 4-state filter decouples
into two identical 2-state scalar filters (one per coordinate), giving
B*2 = 8192 independent scalar sequences.

Blocking time into chunks of C=126 steps turns the filter into one
[128x128] @ [128x512] matmul per block and PSUM chunk: contract rows 0,1
carry the filter state from the previous block, rows 2+j the block's
measurements; output rows 0,1 duplicate the end-of-block state (next block's
carry), rows 2+j the filtered positions.  The Riccati recursion reaches
steady state inside block 0, so only two coefficient matrices exist: block 0
(init folded in, carry coefficients zero — the host zero-fills those rows)
and the shared steady-state matrix used by every other block, including the
short zero-padded last one.  Coefficients are precomputed on the host in
float64.

Layout / engines
----------------
Everything on the wire is float16: the rel-err budget (2e-2) dwarfs the
~3.6e-4 this costs, it halves HBM traffic (the kernel is DMA-bound), and
fp16 matmuls run the PE at full rate.

DMA descriptor generation is serial on the issuing sequencer (~7 ns/desc),
so both z and v live in DRAM as [128, NBLK*NCOLS] SLABS: partition row
2+j holds step j of every block side by side.  One DMA then moves a GROUP
of up to 4 consecutive blocks with just 128 descriptors of contiguous
8 KB lines (vs 126 x 2 KB per block in [time, batch] layout), cutting
trigger-side generation ~4x.  Group sizes ramp 1,1,2,4,... so the first
matmul isn't gated on a full group load.  The host packs/unpacks the slabs.

The serial carry chain stays short: ACT copies the matmul's state rows into
the next block's carry slot ([2 x 512] per chunk) while ALL full-tile
PSUM->SBUF evictions run on DVE, so the chain never queues behind an
eviction.

Sharding: data-parallel across 8 NeuronCores, 512 trajectories (1024 scalar
sequences) per core.
"""

import numpy as np

import concourse.bass as bass
import concourse.bacc as bacc
import concourse.mybir as mybir
from concourse.bass_utils import run_bass_kernel_spmd
from concourse.tile import TileContext

# Problem constants (hardcoded per harness contract).
B = 4096
T = 4096
DT = 1.0
PROCESS_VARIANCE = 1e-05
MEASUREMENT_VARIANCE = 0.1
INIT_ERROR = 1.0

N_CORES = 8
NCOLS = (B * 2) // N_CORES  # 1024 scalar sequences per core
MAIN_C = 126                # steps per block
LAST_C = T - (T // MAIN_C) * MAIN_C  # 64 (block 32, zero-padded to 126)
NBLK = T // MAIN_C + (1 if LAST_C else 0)  # 33
CHUNK = 512                 # matmul moving free-dim (one fp32 PSUM bank)

# blocks per DMA group: ramp in for a fast first matmul, 1-block tail
GSIZES = [1, 1, 2] + [4] * 7 + [1]
assert sum(GSIZES) == NBLK
GBASE = np.cumsum([0] + GSIZES).tolist()
GMAX = max(GSIZES)

DT_F16 = mybir.dt.float16
DT_F32 = mybir.dt.float32


def _riccati():
    F = np.array([[1.0, DT], [0.0, 1.0]], dtype=np.float64)
    I2 = np.eye(2, dtype=np.float64)
    P = INIT_ERROR * I2.copy()
    A = np.zeros((T, 2, 2), dtype=np.float64)
    k = np.zeros((T, 2), dtype=np.float64)
    for t in range(T):
        Pp = F @ P @ F.T + PROCESS_VARIANCE * I2
        s = Pp[0, 0] + MEASUREMENT_VARIANCE
        kt = Pp[:, 0] / s
        k[t] = kt
        KH = np.zeros((2, 2), dtype=np.float64)
        KH[:, 0] = kt
        P = (I2 - KH) @ Pp
        A[t] = (I2 - KH) @ F
    return A, k


def _precompute_u():
    """Returns U [128, 256] f16 in lhsT layout (U[i, m] = coefficient of
    contract input i in output m).  Cols 0:128 = block 0 (init folded, carry
    rows zero), cols 128:256 = steady-state block.  Contract rows 0,1 =
    carry, 2+j = z_j; out cols 0 = p_last, 1 = v_last, 2+j = p_j."""
    A, k = _riccati()
    U = np.zeros((128, 256), dtype=np.float64)
    for sl, t0 in ((0, 0), (1, MAIN_C)):
        L = U[:, sl * 128:(sl + 1) * 128]
        Rc = np.zeros((2, 128), dtype=np.float64)
        if sl == 0:
            Rc[0, 2] = 1.0  # x_{-1} = [z_0, 0]; z_0 is contract input 2
        else:
            Rc[0, 0] = 1.0  # carry row 0 = p_prev
            Rc[1, 1] = 1.0  # carry row 1 = v_prev
        for j in range(MAIN_C):
            t = t0 + j
            Rc = A[t] @ Rc
            Rc[:, 2 + j] += k[t]
            L[:, 2 + j] = Rc[0, :]
        L[:, 0] = Rc[0, :]  # p_last (dup) -> next block carry row 0
        L[:, 1] = Rc[1, :]  # v_last      -> next block carry row 1
    return np.ascontiguousarray(U.astype(np.float16))


def _build_nc():
    nchunks = NCOLS // CHUNK
    ngroups = len(GSIZES)

    nc = bacc.Bacc()
    # group 0's measurements and the coefficients ride one DMA: z0u cols
    # 0:NCOLS = block 0 (carry rows host-zeroed), NCOLS:NCOLS+256 = U
    z0u = nc.dram_tensor("z0u", [128, NCOLS + 256], DT_F16, kind="ExternalInput")
    z = nc.dram_tensor("z", [128, NBLK * NCOLS], DT_F16, kind="ExternalInput")
    v = nc.dram_tensor("v", [128, NBLK * NCOLS], DT_F16, kind="ExternalOutput")

    with TileContext(nc) as tc:
        with (
            tc.tile_pool(name="consts", bufs=1) as cpool,
            tc.tile_pool(name="zpool", bufs=4) as zpool,
            tc.tile_pool(name="vpool", bufs=4) as vpool,
            tc.tile_pool(name="psum", bufs=8, space="PSUM") as ppool,
        ):
            gtiles = {}

            def fetch_group(g):
                # carry rows 0,1 are ACT-written, not loaded (group 0's came
                # from the host inside z0u).  Two groups ride the SWDGE
                # (gpsimd) ring group, which maps to the two DMA engines the
                # HWDGE rings never use.
                gs = GSIZES[g]
                gt = zpool.tile([128, GMAX * NCOLS], DT_F16, tag="zg")
                eng = nc.gpsimd if g in (5, 7) else nc.sync
                eng.dma_start(
                    gt[2:128, 0:gs * NCOLS],
                    z[2:128, GBASE[g] * NCOLS:(GBASE[g] + gs) * NCOLS],
                )
                gtiles[g] = gt

            def ztile_of(b):
                """(tile, col offset) holding block b."""
                g = 0
                while GBASE[g + 1] <= b:
                    g += 1
                return gtiles[g], (b - GBASE[g]) * NCOLS

            g0u = cpool.tile([128, NCOLS + 256], DT_F16)
            nc.sync.dma_start(g0u[:, :], z0u[:, :])
            gtiles[0] = g0u
            u_t = g0u  # coefficient cols live at NCOLS + [0, 256)
            for g in range(1, 5):
                fetch_group(g)

            for g in range(ngroups):
                gs = GSIZES[g]
                vgt = vpool.tile([128, GMAX * NCOLS], DT_F16, tag="vg")
                gout0 = 0  # first block of the group not yet DMA'd out
                for q in range(gs):
                    b = GBASE[g] + q
                    zgt = gtiles[g]
                    usel = bass.ds(NCOLS, 128) if b == 0 else bass.ds(NCOLS + 128, 128)
                    pss = []
                    for ci in range(nchunks):
                        zcols = bass.ds(q * NCOLS + ci * CHUNK, CHUNK)
                        ps = ppool.tile([128, CHUNK], DT_F32)
                        nc.tensor.matmul(
                            ps[:, :], u_t[0:128, usel], zgt[0:128, zcols],
                            start=True, stop=True,
                        )
                        # state rows -> next block's carry slot, on ACT (off
                        # the eviction path so the serial chain stays short)
                        if b + 1 < NBLK:
                            nt, noff = ztile_of(b + 1)
                            nc.scalar.copy(
                                nt[0:2, bass.ds(noff + ci * CHUNK, CHUNK)],
                                ps[0:2, :],
                            )
                        pss.append((ps, zcols))
                    # full-tile evictions all on DVE
                    for ps, cols in pss:
                        nc.vector.tensor_copy(vgt[:, cols], ps[:, :])
                    # drain finished pairs of blocks early so outputs never
                    # bunch up at the end of the run
                    if q == gs - 1 or q == gout0 + 1:
                        c0, c1 = GBASE[g] + gout0, GBASE[g] + q + 1
                        nc.sync.dma_start(
                            v[2:128, c0 * NCOLS:c1 * NCOLS],
                            vgt[2:128, gout0 * NCOLS:(q + 1) * NCOLS],
                        )
                        gout0 = q + 1
                if g + 5 < ngroups:
                    fetch_group(g + 5)
    nc.finalize()
    return nc


_CACHE = {}


def _pack_z(x):
    """[B, T, 2] f32 -> slab [128, NBLK, B*2] f16:
    row 2+j, slab b = measurements at step b*126+j (zero-padded)."""
    zt = x.transpose(1, 0, 2).reshape(T, B * 2).astype(np.float16)
    ztp = np.zeros((NBLK * MAIN_C, B * 2), np.float16)
    ztp[:T] = zt
    slab = np.zeros((128, NBLK, B * 2), np.float16)
    slab[2:128] = ztp.reshape(NBLK, MAIN_C, B * 2).transpose(1, 0, 2)
    return slab


def _unpack_v(v_slab):
    """slab [128, NBLK, B*2] f16 -> [B, T, 2] f32."""
    vt = v_slab[2:128].transpose(1, 0, 2).reshape(NBLK * MAIN_C, B * 2)[:T]
    return np.ascontiguousarray(
        vt.astype(np.float32).reshape(T, B, 2).transpose(1, 0, 2))


def _run(x_seq: np.ndarray, trace: bool = False):
    if "nc" not in _CACHE:
        _CACHE["nc"] = _build_nc()
        _CACHE["u"] = _precompute_u()
    nc = _CACHE["nc"]
    u_all = _CACHE["u"]

    x = np.asarray(x_seq)
    assert x.shape == (B, T, 2), x.shape

    slab = _pack_z(x)
    in_maps = []
    for i in range(N_CORES):
        zi = np.ascontiguousarray(
            slab[:, :, i * NCOLS:(i + 1) * NCOLS]).reshape(128, NBLK * NCOLS)
        z0u = np.concatenate([zi[:, 0:NCOLS], u_all], axis=1)
        in_maps.append({"z": zi, "z0u": np.ascontiguousarray(z0u)})
    res = run_bass_kernel_spmd(nc, in_maps, core_ids=list(range(N_CORES)), trace=trace)

    v_slab = np.concatenate(
        [r["v"].reshape(128, NBLK, NCOLS) for r in res.results], axis=2)
    return _unpack_v(v_slab), res


def kernel(x_seq: np.ndarray) -> np.ndarray:
    out, _ = _run(x_seq, trace=False)
    return out
